# revision 22
# baseline (speedup 1.0000x reference)
"""Trainium2 Bass kernel: single-head attention (B=4, N=2048, D=1024).

Sharding: 8 cores = (batch b, query-half h). Each core computes attention for
its 1024 queries against all 2048 keys of its batch. K/VW projections are
deduplicated: each core projects K/VW only for its own 1024 keys, then core
pairs (2b, 2b+1) AllGather the halves (global key order).

Output-projection fusion (associativity): (softmax @ V) @ Wo =
softmax @ (x @ (Wv @ Wo)). The host folds Wvo = Wv @ Wo once (f32, D x D);
the device projects VW = x @ Wvo instead of V, and the attention-weight
matmul then produces the final (unnormalized) output directly -- the
entire 128-pass output projection phase disappears. Per-core device work
drops from 8.6 to 7.5 G-MACs.

Layouts avoid all on-device transposes:
  - QT, KTh produced in [e, n] layout     (lhsT = W as stored, rhs = x^T)
  - VWh produced in [key, e'] layout      (lhsT = x^T block,  rhs = Wvo)
  - scores computed transposed ST[key, q] (lhsT = KT block,   rhs = QT)
  - output outT[e', q]                    (lhsT = VW block,   rhs = UT)
Host transposes x on the way in and outT on the way out.

Precision plan (rel-err budget 2e-2; this config runs at 1.346e-2):
  - bf16 for x, W, VW, UT (fp16 measured 25% SLOWER per matmul on HW)
  - QT/KT stored as fp8 e4m3 scaled by 16; the scores matmul runs in
    MatmulPerfMode.DoubleRow (2 k-subtiles per pass). Each DR stationary
    block is reused for both q-tiles to amortize the 256-col LDWEIGHTS.
    The 1/(16*16*sqrt(N)) descale folds into the exp activation scale.
  - fp8 Q/K *projections* were tried and rejected: they push rel err to
    2.3e-2 (over budget) -- QT/KT fp8 quantization already dominates the
    error budget (1.27e-2 of 1.34e-2 RMS).
  - mask stays additive bf16 (-1e9); fp8 K AllGather; f16 output store
    (unnormalized |out| <= ~80, far from f16 max).

Softmax: scores are O(+-3) so exp() without max-subtraction is safe in f32.
NORMALIZATION IS DONE ON THE HOST: the device ships the unnormalized
output outT = UT @ VW plus the per-query row sums srow; the host divides
and adds bo_eff = bo + bv @ Wo. This removes the [128,512] DVE reciprocal
(3.4us, was on the PE critical path and triggered a HAM half-clock
throttle) and the 1/s broadcast matmuls. Row sums are plain ones-vector
matmul chains over UT (32 passes, 6.9us). Two replacements were tried
and rejected: a DVE adder-tree (the tile scheduler hoists the dependent
sums matmuls ahead of the output chains and the PE stalls 20us on the
tree) and gpsimd tensor_reduce(axis=C) (CROSS_LANE_REDUCE on [128,1024]
f32 measures 128us -- ~125ns/element -- and gpsimd tensor_tensor is
2.1us per [128,1024] op, 1.75x slower than DVE).

Collectives (pair AllGathers on the serial cc stream; ~20us fixed cost
each): one K gather (fp8, 1MB) + VW in TWO e-column chunks (bf16, 1MB
each) so the first four output chains unblock one chunk earlier. Stream
order = consumption order: K (scores), VWa, VWb. Gather loads are single
big DMAs (32 -> 8 descriptor-gen slots) on the sync queue; they must NOT
ride the gpsimd queue (collective_compute blocks it until the cc stream
accepts the op) and the first output phase's stores are routed to the
Activation queue so the gated VWb loads fire the moment the gather lands.

DMA queues: startup x0 loads ride the SP queue while wk rides the
Activation queue (parallel HWDGE paths; the Activation queue must stay
short or it head-of-line-blocks the K-proj PSUM-drain ACTIVATEs --
measured 11.5us PE stall + HAM half-clock). x1 rides the gpsimd SWDGE
queue, which is idle until the first kth store. Output stores: qt0 ->
Activation queue, qt1 -> SP queue.

Measured (full clock, 216ns per 512-col pass): 198.3us vs 237.3us
baseline (-16.4%). PE busy 176.5us vs 169.3us ideal for 784 passes; the
rest is fixed NEFF init (7.4us), DR LDWEIGHTS overhead (~7us), row sums
(6.9us), teardown barriers (~4us). The device DVFS alternates between
~2.37GHz and ~1.98GHz states run to run (259ns pass spacing = slow
state); compare runs by median matmul spacing, not wall time.
"""

import sys

if "/opt/trn_rl_repo" not in sys.path:
    sys.path.insert(0, "/opt/trn_rl_repo")

import numpy as np
import ml_dtypes

B, N, D = 4, 2048, 1024
P = 128
NQ = N // 2          # queries (and locally-projected keys) per core
DB = D // P          # 8   d/e blocks of 128
KB = N // P          # 16  key blocks of 128
KBH = NQ // P        # 8   key blocks per half
FT = 512             # matmul free-dim tile
NTH = NQ // FT       # 2   n tiles over own half
QT_TILES = NQ // FT  # 2   q tiles
ET = D // FT         # 2   e tiles

BF16H = ml_dtypes.bfloat16
F8 = ml_dtypes.float8_e4m3  # TRN fp8: max normal 240
MASK_NEG = -1.0e9
SQ = 16.0            # QT = fp8(SQ * Q)
SK = 16.0            # KT = fp8(SK * K)
EXP_SCALE = float(1.0 / (SQ * SK * np.sqrt(np.float64(N))))

_cached = None


def _build_program():
    import concourse.bacc as bacc
    import concourse.mybir as mybir
    import concourse.tile as tile

    f32 = mybir.dt.float32
    f32r = mybir.dt.float32r
    f16 = mybir.dt.float16
    bf16 = mybir.dt.bfloat16
    fp8 = mybir.dt.float8e4
    AF = mybir.ActivationFunctionType
    ALU = mybir.AluOpType
    DR = mybir.MatmulPerfMode.DoubleRow
    PAIRS = [[0, 1], [2, 3], [4, 5], [6, 7]]

    nc = bacc.Bacc("TRN2", target_bir_lowering=False, debug=False, num_devices=8)

    xTh = nc.dram_tensor("xTh", [D, NQ], bf16, kind="ExternalInput").ap()
    maskadd = nc.dram_tensor("maskadd", [N, NQ], bf16, kind="ExternalInput").ap()
    wq = nc.dram_tensor("wq", [D, D], bf16, kind="ExternalInput").ap()
    wk = nc.dram_tensor("wk", [D, D], bf16, kind="ExternalInput").ap()
    wvo = nc.dram_tensor("wvo", [D, D], bf16, kind="ExternalInput").ap()
    bq = nc.dram_tensor("bq", [D], f32, kind="ExternalInput").ap()
    outT = nc.dram_tensor("outT", [D, NQ], f16, kind="ExternalOutput").ap()
    srow = nc.dram_tensor("srow", [1, NQ], f32, kind="ExternalOutput").ap()

    xTh_r = xTh.rearrange("(db p) n -> p db n", p=P)
    wq_r = wq.rearrange("(db p) e -> p db e", p=P)
    wk_r = wk.rearrange("(db p) e -> p db e", p=P)
    wvo_r = wvo.rearrange("(db p) e -> p db e", p=P)
    bq_r = bq.rearrange("(eb p) -> p eb", p=P)

    with tile.TileContext(nc) as tc:
        with (
            tc.tile_pool(name="persist", bufs=1) as persist,
            tc.tile_pool(name="dram", bufs=1, space="DRAM") as dram,
        ):
            ones_kb = persist.tile([P, 1], bf16)
            nc.vector.memset(ones_kb, 1.0)

            QT = persist.tile([P, DB, NQ], fp8)
            KT = persist.tile([P, DB, N], fp8)
            # VW split by gather half so the output matmul's first chains
            # can start as soon as gather-half-0 lands.
            VW01 = [persist.tile([P, KBH, D], bf16, name=f"VW{g}")
                    for g in range(2)]
            maskfull = persist.tile([P, KB, NQ], bf16)

            # Single K AllGather (Q-proj runs between K-proj and the
            # scores phase, so K has ~30us of slack; one gather keeps the
            # serial cc stream short so the VW chunks land early).
            kth_d = dram.tile([D, NQ], fp8)
            ktg_d = dram.tile([2, D, NQ], fp8)
            vha_d = dram.tile([NQ, FT], bf16)
            vhb_d = dram.tile([NQ, FT], bf16)
            vga_d = dram.tile([2, NQ, FT], bf16)
            vgb_d = dram.tile([2, NQ, FT], bf16)

            # ---- Phase A: project K half, VW half (collectives), then Q ----
            with (
                tc.tile_pool(name="wpool", bufs=1) as wpool,
                tc.tile_pool(name="xpool", bufs=1) as xpool,
            ):
                # All are distinct named tiles (live through phase A) --
                # tag rotation here deadlocks the in-order sync DMA queue.
                # wk split into column halves: pass 1 of K-proj only
                # touches ebs 0-3 (cols 0:512), so the first-pass DMA
                # demand halves and the PE outruns the queue less.
                wkA_ts = []
                for db in range(DB):
                    wkt = wpool.tile([P, FT], bf16, name=f"wkA{db}")
                    nc.sync.dma_start(out=wkt, in_=wk_r[:, db, 0:FT])
                    wkA_ts.append(wkt)
                x0_ts = []
                for db in range(DB):
                    x0t = xpool.tile([P, FT], bf16, name=f"x0_{db}")
                    nc.scalar.dma_start(out=x0t, in_=xTh_r[:, db, 0:FT])
                    x0_ts.append(x0t)
                wkB_ts = []
                for db in range(DB):
                    wkt = wpool.tile([P, FT], bf16, name=f"wkB{db}")
                    nc.sync.dma_start(out=wkt, in_=wk_r[:, db, FT:D])
                    wkB_ts.append(wkt)

                def wk_slice(db, eb):
                    if eb < DB // 2:
                        return wkA_ts[db][:, eb * P : (eb + 1) * P]
                    return wkB_ts[db][:, (eb - DB // 2) * P : (eb - DB // 2 + 1) * P]

                x1_ts = []
                for db in range(DB):
                    x1t = xpool.tile([P, FT], bf16, name=f"x1_{db}")
                    nc.gpsimd.dma_start(out=x1t, in_=xTh_r[:, db, FT:NQ])
                    x1_ts.append(x1t)
                x_ts = [x0_ts, x1_ts]
                wvo_t = wpool.tile([P, DB, D], bf16)
                for db in range(DB):
                    nc.sync.dma_start(out=wvo_t[:, db, :], in_=wvo_r[:, db, :])
                wq_t = wpool.tile([P, DB, D], bf16)
                for db in range(DB):
                    nc.sync.dma_start(out=wq_t[:, db, :], in_=wq_r[:, db, :])
                bq_t = persist.tile([P, DB], f32)
                nc.sync.dma_start(out=bq_t, in_=bq_r)
                # Whole mask loads during the collective-latency window; it
                # must precede the gather loads in sync-queue order so it is
                # not trapped behind their semaphore waits.
                maskadd_r = maskadd.rearrange("(kb p) n -> p kb n", p=P)
                for qt in range(QT_TILES):
                    nc.sync.dma_start(
                        out=maskfull[:, :, qt * FT : (qt + 1) * FT],
                        in_=maskadd_r[:, :, qt * FT : (qt + 1) * FT],
                    )

                # K projection (own half). First n-tile is db-outer so the
                # first matmul only needs one wk chunk + one x chunk
                # (8 concurrent PSUM groups); the rest uses a 4-buf pool.
                stage_ctx = tc.tile_pool(name="stage", bufs=1)
                stage = stage_ctx.__enter__()
                kth_t = stage.tile([P, DB, NQ], fp8)
                vh_t = stage.tile([P, KBH, D], bf16)
                # psA (4 banks) opens first so the eb-outer chains flow
                # through one rotation with no pool-transition stall;
                # psK0 (4 banks, pass-1 only) nests inside and exits while
                # pass 2 runs.
                psA_ctx = tc.tile_pool(name="psA", bufs=4, space="PSUM")
                psA = psA_ctx.__enter__()
                with tc.tile_pool(name="psK0", bufs=1, space="PSUM") as psK:
                    pss = [
                        psK.tile([P, FT], f32, name=f"psk0_{eb}")
                        for eb in range(DB // 2)
                    ]
                    # Pass 1 (ebs 0-3) db-outer: first matmul needs only
                    # wk0+x0_0 (4 concurrent PSUM accumulation groups).
                    for db in range(DB):
                        for eb in range(DB // 2):
                            nc.tensor.matmul(
                                pss[eb],
                                lhsT=wk_slice(db, eb),
                                rhs=x0_ts[db],
                                start=(db == 0),
                                stop=(db == DB - 1),
                            )
                    for eb in range(DB // 2):
                        nc.scalar.activation(
                            out=kth_t[:, eb, 0:FT], in_=pss[eb],
                            func=AF.Identity, scale=SK,
                        )
                # Pass 2 (ebs 4-7) eb-outer on the psA rotation.
                for eb in range(DB // 2, DB):
                    ps = psA.tile([P, FT], f32, tag="ps")
                    for db in range(DB):
                        nc.tensor.matmul(
                            ps,
                            lhsT=wk_slice(db, eb),
                            rhs=x0_ts[db],
                            start=(db == 0),
                            stop=(db == DB - 1),
                        )
                    nc.scalar.activation(
                        out=kth_t[:, eb, 0:FT], in_=ps,
                        func=AF.Identity, scale=SK,
                    )

                # K projection n1 -> second K chunk gather
                for eb in range(DB):
                    ps = psA.tile([P, FT], f32, tag="ps")
                    for db in range(DB):
                        nc.tensor.matmul(
                            ps,
                            lhsT=wk_slice(db, eb),
                            rhs=x1_ts[db],
                            start=(db == 0),
                            stop=(db == DB - 1),
                        )
                    nc.scalar.activation(
                        out=kth_t[:, eb, FT:NQ], in_=ps,
                        func=AF.Identity, scale=SK,
                    )
                kth_dr = kth_d.rearrange("(db p) n -> p db n", p=P)
                nc.gpsimd.dma_start(out=kth_dr[:, :, :], in_=kth_t[:, :, :])
                nc.gpsimd.collective_compute(
                    "AllGather",
                    ALU.bypass,
                    replica_groups=PAIRS,
                    ins=[kth_d[:, :]],
                    outs=[ktg_d[:, :, :]],
                )

                # VW projection (own half) -> staging -> DRAM -> AllGather.
                # Last on the serial cc stream: with the fused output
                # matmul, VW is first consumed ~34us after st starts --
                # the K chunk gathers must land first (st(1) stalls 10.6us
                # if the 34us VW gather cuts ahead of K chunk B).
                for ks in range(KBH):
                    nt, kso = divmod(ks, FT // P)
                    for et in range(ET):
                        esl = slice(et * FT, (et + 1) * FT)
                        ps = psA.tile([P, FT], f32, tag="ps")
                        for db in range(DB):
                            nc.tensor.matmul(
                                ps,
                                lhsT=x_ts[nt][db][:, kso * P : (kso + 1) * P],
                                rhs=wvo_t[:, db, esl],
                                start=(db == 0),
                                stop=(db == DB - 1),
                            )
                        nc.any.tensor_copy(out=vh_t[:, ks, esl], in_=ps)
                # VW exchanged as two e-column chunk AllGathers: the fused
                # output matmul consumes e-blocks in order, so chunk A
                # (e 0:512) unblocks its first four chains ~17us earlier.
                vha_dr = vha_d.rearrange("(kb p) e -> p kb e", p=P)
                nc.gpsimd.dma_start(out=vha_dr[:, :, :], in_=vh_t[:, :, 0:FT])
                nc.gpsimd.collective_compute(
                    "AllGather",
                    ALU.bypass,
                    replica_groups=PAIRS,
                    ins=[vha_d[:, :]],
                    outs=[vga_d[:, :, :]],
                )
                vhb_dr = vhb_d.rearrange("(kb p) e -> p kb e", p=P)
                nc.gpsimd.dma_start(out=vhb_dr[:, :, :], in_=vh_t[:, :, FT:D])
                nc.gpsimd.collective_compute(
                    "AllGather",
                    ALU.bypass,
                    replica_groups=PAIRS,
                    ins=[vhb_d[:, :]],
                    outs=[vgb_d[:, :, :]],
                )
                # Gather loads stay on the sync queue, after the mask in
                # program order. (gpsimd was tried: collective_compute
                # blocks that queue until the serial cc stream accepts it,
                # so loads behind a trigger inherit the previous gather's
                # latency -- 13us st stall. The sync queue is kept free
                # during the first output phase by routing its stores to
                # the Activation queue instead.)
                for g in range(2):
                    kg_gr = ktg_d[g].rearrange("(db p) n -> p db n", p=P)
                    nc.sync.dma_start(
                        out=KT[:, :, g * NQ : (g + 1) * NQ],
                        in_=kg_gr[:, :, :],
                    )
                for et, vg in enumerate([vga_d, vgb_d]):
                    esl = slice(et * FT, (et + 1) * FT)
                    for g in range(2):
                        vg_gr = vg[g].rearrange("(kb p) e -> p kb e", p=P)
                        nc.sync.dma_start(
                            out=VW01[g][:, :, esl],
                            in_=vg_gr[:, :, :],
                        )

                # Q projection (overlaps the collectives)
                for nt in range(NTH):
                    nsl = slice(nt * FT, (nt + 1) * FT)
                    for eb in range(DB):
                        ps = psA.tile([P, FT], f32, tag="ps")
                        for db in range(DB):
                            nc.tensor.matmul(
                                ps,
                                lhsT=wq_t[:, db, eb * P : (eb + 1) * P],
                                rhs=x_ts[nt][db],
                                start=(db == 0),
                                stop=(db == DB - 1),
                            )
                        nc.scalar.activation(
                            out=QT[:, eb, nsl],
                            in_=ps,
                            func=AF.Identity,
                            bias=bq_t[:, eb : eb + 1],
                            scale=SQ,
                        )
                psA_ctx.__exit__(None, None, None)
                stage_ctx.__exit__(None, None, None)

            # ---- Phase B: scores+softmax, then fused output matmul ----
            with tc.tile_pool(name="persist2", bufs=1) as persist2:
                UT = persist2.tile([P, KB, NQ], bf16)
                s_sb = persist2.tile([1, NQ], f32)

                with (
                    tc.tile_pool(name="scr", bufs=4) as scr,
                    tc.tile_pool(name="psST", bufs=6, space="PSUM") as psST,
                    tc.tile_pool(name="psAV", bufs=2, space="PSUM") as psAV,
                ):
                    # Consume keys in K-gather arrival order:
                    # gather 0 -> kb {0-3, 8-11}, gather 1 -> kb {4-7, 12-15}
                    ST_KB_HALVES = [[0, 1, 2, 3, 8, 9, 10, 11],
                                    [4, 5, 6, 7, 12, 13, 14, 15]]

                    def st_half(half):
                        # Both q-tiles per kb so each DR stationary load
                        # (KT e-pair block) serves two moving matmuls.
                        for kb in ST_KB_HALVES[half]:
                            pss_qt = [
                                psST.tile([P, FT], f32, tag="st",
                                         name=f"ps_st{qt}_{kb}")
                                for qt in range(QT_TILES)
                            ]
                            for ep in range(DB // 2):
                                for qt in range(QT_TILES):
                                    qsl = slice(qt * FT, (qt + 1) * FT)
                                    nc.tensor.matmul(
                                        pss_qt[qt],
                                        lhsT=KT[:, 2 * ep : 2 * ep + 2,
                                                kb * P : (kb + 1) * P],
                                        rhs=QT[:, 2 * ep : 2 * ep + 2, qsl],
                                        start=(ep == 0),
                                        stop=(ep == DB // 2 - 1),
                                        perf_mode=DR,
                                    )
                            for qt in range(QT_TILES):
                                qsl = slice(qt * FT, (qt + 1) * FT)
                                sc = scr.tile([P, FT], f32, tag="sc",
                                              name=f"sc{qt}_{kb}")
                                nc.vector.tensor_tensor(
                                    sc, pss_qt[qt], maskfull[:, kb, qsl],
                                    op=ALU.add
                                )
                                nc.scalar.activation(
                                    out=UT[:, kb, qsl], in_=sc, func=AF.Exp,
                                    scale=EXP_SCALE,
                                )

                    def sums(qt):
                        # Row sums via ones-vector matmul over UT's key
                        # partitions. (A DVE adder-tree variant was tried:
                        # the scheduler hoists these matmuls ahead of the
                        # output chains and the PE then stalls 19.8us on
                        # the 18us serial tree. UT is ready by construction,
                        # so the plain version can never stall.)
                        qsl = slice(qt * FT, (qt + 1) * FT)
                        pss = psST.tile([1, FT], f32, tag="st",
                                        name=f"pss{qt}")
                        for kb in range(KB):
                            nc.tensor.matmul(
                                pss,
                                lhsT=ones_kb,
                                rhs=UT[:, kb, qsl],
                                start=(kb == 0),
                                stop=(kb == KB - 1),
                            )
                        nc.vector.tensor_copy(out=s_sb[:, qsl], in_=pss)

                    def avw(qt):
                        # Fused unnormalized output: outT = VW^T-blocks @ UT.
                        # Normalization happens on the host (srow output).
                        qsl = slice(qt * FT, (qt + 1) * FT)
                        for eb in range(DB):
                            ps = psAV.tile([P, FT], f32, tag="av",
                                           name=f"ps_av{qt}_{eb}")
                            for kb in range(KB):
                                nc.tensor.matmul(
                                    ps,
                                    lhsT=VW01[kb // KBH][:, kb % KBH,
                                                        eb * P : (eb + 1) * P],
                                    rhs=UT[:, kb, qsl],
                                    start=(kb == 0),
                                    stop=(kb == KB - 1),
                                )
                            ot = scr.tile([P, FT], f16, tag="ot")
                            nc.scalar.activation(
                                out=ot, in_=ps, func=AF.Identity, scale=1.0,
                            )
                            # qt0 stores all go to the Activation queue so
                            # the sync queue is empty when the gated VWb
                            # loads fire mid-avw(0); qt1 uses sync.
                            eng = nc.scalar if qt == 0 else nc.sync
                            qsl_o = slice(qt * FT, (qt + 1) * FT)
                            eng.dma_start(
                                out=outT[eb * P : (eb + 1) * P, qsl_o],
                                in_=ot,
                            )

                    # Order: the DVE tree fills during avw(0); the two sums
                    # passes slot between the output chains; srow's store
                    # issues before avw(1)'s drains hit the scalar queue.
                    st_half(0)
                    st_half(1)
                    avw(0)
                    avw(1)
                    sums(0)
                    sums(1)
                    nc.scalar.dma_start(out=srow, in_=s_sb)

    nc.compile()
    return nc


def _get_program():
    global _cached
    if _cached is None:
        _cached = _build_program()
    return _cached


def make_in_maps(x, mask, Wq, bq, Wk, bk, Wv, bv, Wo, bo):
    """Host-side preprocessing: per-core input dicts + bo_eff."""
    wq_h = Wq.astype(BF16H)
    wk_h = Wk.astype(BF16H)
    wvo_h = (Wv.astype(np.float32) @ Wo.astype(np.float32)).astype(BF16H)
    bq_s = (bq.astype(np.float32) * np.float32(SQ)).astype(np.float32)
    bo_eff = (
        bo.astype(np.float64) + bv.astype(np.float64) @ Wo.astype(np.float64)
    ).astype(np.float32)

    in_maps = []
    for c in range(8):
        b, h = divmod(c, 2)
        qs = slice(h * NQ, (h + 1) * NQ)
        xTh_c = np.ascontiguousarray(x[b, qs].T).astype(BF16H)  # [D, NQ]
        madd = np.where(
            mask[b, qs, :].T, np.float32(MASK_NEG), np.float32(0.0)
        ).astype(BF16H)  # [N, NQ], global key order
        in_maps.append(
            {
                "xTh": xTh_c,
                "maskadd": np.ascontiguousarray(madd),
                "wq": wq_h,
                "wk": wk_h,
                "wvo": wvo_h,
                "bq": bq_s,
            }
        )
    return in_maps, bo_eff


def assemble(results, bo_eff):
    out = np.empty((B, N, D), dtype=np.float32)
    for c in range(8):
        b, h = divmod(c, 2)
        s = results[c]["srow"].reshape(-1).astype(np.float32)  # [NQ]
        o = results[c]["outT"].T.astype(np.float32)            # [NQ, D]
        out[b, h * NQ : (h + 1) * NQ, :] = o / s[:, None] + bo_eff
    return out


def kernel(x, mask, Wq, bq, Wk, bk, Wv, bv, Wo, bo):
    from concourse.bass_utils import run_bass_kernel_spmd

    nc = _get_program()
    x, mask, Wq, bq, Wk, bk, Wv, bv, Wo, bo = (
        np.asarray(a) for a in (x, mask, Wq, bq, Wk, bk, Wv, bv, Wo, bo)
    )
    in_maps, bo_eff = make_in_maps(x, mask, Wq, bq, Wk, bk, Wv, bv, Wo, bo)
    res = run_bass_kernel_spmd(nc, in_maps, list(range(8)))
    return assemble(res.results, bo_eff)


# revision 23
# speedup vs baseline: 1.0164x; 1.0164x over previous
"""Trainium2 Bass kernel: single-head attention (B=4, N=2048, D=1024).

Sharding: 8 cores = (batch b, query-half h). Each core computes attention for
its 1024 queries against all 2048 keys of its batch. K/VW projections are
deduplicated: each core projects K/VW only for its own 1024 keys, then core
pairs (2b, 2b+1) AllGather the halves (global key order).

Output-projection fusion (associativity): (softmax @ V) @ Wo =
softmax @ (x @ (Wv @ Wo)). The host folds Wvo = Wv @ Wo once (f32, D x D);
the device projects VW = x @ Wvo instead of V, and the attention-weight
matmul then produces the final (unnormalized) output directly -- the
entire 128-pass output projection phase disappears. Per-core device work
drops from 8.6 to 7.5 G-MACs.

Layouts avoid all on-device transposes:
  - QT, KTh produced in [e, n] layout     (lhsT = W as stored, rhs = x^T)
  - VWh produced in [key, e'] layout      (lhsT = x^T block,  rhs = Wvo)
  - scores computed transposed ST[key, q] (lhsT = KT block,   rhs = QT)
  - output outT[e', q]                    (lhsT = VW block,   rhs = UT)
Host transposes x on the way in and outT on the way out.

Precision plan (rel-err budget 2e-2; this config runs at 1.346e-2):
  - bf16 for x, W, VW, UT (fp16 measured 25% SLOWER per matmul on HW)
  - QT/KT stored as fp8 e4m3 scaled by 16; the scores matmul runs in
    MatmulPerfMode.DoubleRow (2 k-subtiles per pass). Each DR stationary
    block is reused for both q-tiles to amortize the 256-col LDWEIGHTS.
    The 1/(16*16*sqrt(N)) descale folds into the exp activation scale.
  - fp8 Q/K *projections* were tried and rejected: they push rel err to
    2.3e-2 (over budget) -- QT/KT fp8 quantization already dominates the
    error budget (1.27e-2 of 1.34e-2 RMS).
  - mask stays additive bf16 (-1e9); fp8 K AllGather; f16 output store
    (unnormalized |out| <= ~80, far from f16 max).

Softmax: scores are O(+-3) so exp() without max-subtraction is safe in f32.
NORMALIZATION IS DONE ON THE HOST: the device ships the unnormalized
output outT = UT @ VW plus the per-query row sums srow; the host divides
and adds bo_eff = bo + bv @ Wo. This removes the [128,512] DVE reciprocal
(3.4us, was on the PE critical path and triggered a HAM half-clock
throttle) and the 1/s broadcast matmuls. Row sums are plain ones-vector
matmul chains over UT (32 passes, 6.9us). Two replacements were tried
and rejected: a DVE adder-tree (the tile scheduler hoists the dependent
sums matmuls ahead of the output chains and the PE stalls 20us on the
tree) and gpsimd tensor_reduce(axis=C) (CROSS_LANE_REDUCE on [128,1024]
f32 measures 128us -- ~125ns/element -- and gpsimd tensor_tensor is
2.1us per [128,1024] op, 1.75x slower than DVE).

Collectives (pair AllGathers on the serial cc stream; ~20us fixed cost
each): one K gather (fp8, 1MB) + VW in TWO e-column chunks (bf16, 1MB
each) so the first four output chains unblock one chunk earlier. Stream
order = consumption order: K (scores), VWa, VWb. Gather loads are single
big DMAs (32 -> 8 descriptor-gen slots) on the sync queue; they must NOT
ride the gpsimd queue (collective_compute blocks it until the cc stream
accepts the op) and the first output phase's stores are routed to the
Activation queue so the gated VWb loads fire the moment the gather lands.

DMA queues: startup x0 loads ride the SP queue while wk rides the
Activation queue (parallel HWDGE paths; the Activation queue must stay
short or it head-of-line-blocks the K-proj PSUM-drain ACTIVATEs --
measured 11.5us PE stall + HAM half-clock). x1 rides the gpsimd SWDGE
queue, which is idle until the first kth store. Output stores: qt0 ->
Activation queue, qt1 -> SP queue.

Measured (full clock, 216ns per 512-col pass): 198.3us vs 237.3us
baseline (-16.4%). PE busy 176.5us vs 169.3us ideal for 784 passes; the
rest is fixed NEFF init (7.4us), DR LDWEIGHTS overhead (~7us), row sums
(6.9us), teardown barriers (~4us). The device DVFS alternates between
~2.37GHz and ~1.98GHz states run to run (259ns pass spacing = slow
state); compare runs by median matmul spacing, not wall time.
"""

import sys

if "/opt/trn_rl_repo" not in sys.path:
    sys.path.insert(0, "/opt/trn_rl_repo")

import numpy as np
import ml_dtypes

B, N, D = 4, 2048, 1024
P = 128
NQ = N // 2          # queries (and locally-projected keys) per core
DB = D // P          # 8   d/e blocks of 128
KB = N // P          # 16  key blocks of 128
KBH = NQ // P        # 8   key blocks per half
FT = 512             # matmul free-dim tile
NTH = NQ // FT       # 2   n tiles over own half
QT_TILES = NQ // FT  # 2   q tiles
ET = D // FT         # 2   e tiles

BF16H = ml_dtypes.bfloat16
F8 = ml_dtypes.float8_e4m3  # TRN fp8: max normal 240
MASK_NEG = -1.0e9
SQ = 16.0            # QT = fp8(SQ * Q)
SK = 16.0            # KT = fp8(SK * K)
EXP_SCALE = float(1.0 / (SQ * SK * np.sqrt(np.float64(N))))

_cached = None


def _build_program():
    import concourse.bacc as bacc
    import concourse.mybir as mybir
    import concourse.tile as tile

    f32 = mybir.dt.float32
    f32r = mybir.dt.float32r
    f16 = mybir.dt.float16
    bf16 = mybir.dt.bfloat16
    fp8 = mybir.dt.float8e4
    AF = mybir.ActivationFunctionType
    ALU = mybir.AluOpType
    DR = mybir.MatmulPerfMode.DoubleRow
    PAIRS = [[0, 1], [2, 3], [4, 5], [6, 7]]

    nc = bacc.Bacc("TRN2", target_bir_lowering=False, debug=False, num_devices=8)

    xTh = nc.dram_tensor("xTh", [D, NQ], bf16, kind="ExternalInput").ap()
    maskadd = nc.dram_tensor("maskadd", [N, NQ], bf16, kind="ExternalInput").ap()
    wq = nc.dram_tensor("wq", [D, D], bf16, kind="ExternalInput").ap()
    wk = nc.dram_tensor("wk", [D, D], bf16, kind="ExternalInput").ap()
    wvo = nc.dram_tensor("wvo", [D, D], bf16, kind="ExternalInput").ap()
    bq = nc.dram_tensor("bq", [D], f32, kind="ExternalInput").ap()
    outT = nc.dram_tensor("outT", [D, NQ], f16, kind="ExternalOutput").ap()
    srow = nc.dram_tensor("srow", [1, NQ], f32, kind="ExternalOutput").ap()

    xTh_r = xTh.rearrange("(db p) n -> p db n", p=P)
    wq_r = wq.rearrange("(db p) e -> p db e", p=P)
    wk_r = wk.rearrange("(db p) e -> p db e", p=P)
    wvo_r = wvo.rearrange("(db p) e -> p db e", p=P)
    bq_r = bq.rearrange("(eb p) -> p eb", p=P)

    with tile.TileContext(nc) as tc:
        with (
            tc.tile_pool(name="persist", bufs=1) as persist,
            tc.tile_pool(name="dram", bufs=1, space="DRAM") as dram,
        ):
            ones_kb = persist.tile([P, 1], bf16)
            nc.vector.memset(ones_kb, 1.0)

            QT = persist.tile([P, DB, NQ], fp8)
            KT = persist.tile([P, DB, N], fp8)
            # VW split by gather half so the output matmul's first chains
            # can start as soon as gather-half-0 lands.
            VW01 = [persist.tile([P, KBH, D], bf16, name=f"VW{g}")
                    for g in range(2)]
            maskfull = persist.tile([P, KB, NQ], bf16)

            # Single K AllGather (Q-proj runs between K-proj and the
            # scores phase, so K has ~30us of slack; one gather keeps the
            # serial cc stream short so the VW chunks land early).
            kth_d = dram.tile([D, NQ], fp8)
            ktg_d = dram.tile([2, D, NQ], fp8)
            vha_d = dram.tile([NQ, FT], bf16)
            vhb_d = dram.tile([NQ, FT], bf16)
            vga_d = dram.tile([2, NQ, FT], bf16)
            vgb_d = dram.tile([2, NQ, FT], bf16)

            # ---- Phase A: project K half, VW half (collectives), then Q ----
            with (
                tc.tile_pool(name="wpool", bufs=1) as wpool,
                tc.tile_pool(name="xpool", bufs=1) as xpool,
            ):
                # All are distinct named tiles (live through phase A) --
                # tag rotation here deadlocks the in-order sync DMA queue.
                wk_ts = []
                for db in range(DB):
                    wkt = wpool.tile([P, D], bf16, name=f"wk{db}")
                    nc.scalar.dma_start(out=wkt, in_=wk_r[:, db, :])
                    wk_ts.append(wkt)
                x0_ts = []
                for db in range(DB):
                    x0t = xpool.tile([P, FT], bf16, name=f"x0_{db}")
                    nc.sync.dma_start(out=x0t, in_=xTh_r[:, db, 0:FT])
                    x0_ts.append(x0t)

                def wk_slice(db, eb):
                    return wk_ts[db][:, eb * P : (eb + 1) * P]

                x1_ts = []
                for db in range(DB):
                    x1t = xpool.tile([P, FT], bf16, name=f"x1_{db}")
                    nc.gpsimd.dma_start(out=x1t, in_=xTh_r[:, db, FT:NQ])
                    x1_ts.append(x1t)
                x_ts = [x0_ts, x1_ts]
                wvo_t = wpool.tile([P, DB, D], bf16)
                for db in range(DB):
                    nc.sync.dma_start(out=wvo_t[:, db, :], in_=wvo_r[:, db, :])
                wq_t = wpool.tile([P, DB, D], bf16)
                for db in range(DB):
                    nc.sync.dma_start(out=wq_t[:, db, :], in_=wq_r[:, db, :])
                bq_t = persist.tile([P, DB], f32)
                nc.sync.dma_start(out=bq_t, in_=bq_r)
                # Whole mask loads during the collective-latency window; it
                # must precede the gather loads in sync-queue order so it is
                # not trapped behind their semaphore waits.
                maskadd_r = maskadd.rearrange("(kb p) n -> p kb n", p=P)
                for qt in range(QT_TILES):
                    nc.sync.dma_start(
                        out=maskfull[:, :, qt * FT : (qt + 1) * FT],
                        in_=maskadd_r[:, :, qt * FT : (qt + 1) * FT],
                    )

                # K projection (own half). First n-tile is db-outer so the
                # first matmul only needs one wk chunk + one x chunk
                # (8 concurrent PSUM groups); the rest uses a 4-buf pool.
                stage_ctx = tc.tile_pool(name="stage", bufs=1)
                stage = stage_ctx.__enter__()
                kth_t = stage.tile([P, DB, NQ], fp8)
                vh_t = stage.tile([P, KBH, D], bf16)
                # psA (4 banks) opens first so the eb-outer chains flow
                # through one rotation with no pool-transition stall;
                # psK0 (4 banks, pass-1 only) nests inside and exits while
                # pass 2 runs.
                psA_ctx = tc.tile_pool(name="psA", bufs=4, space="PSUM")
                psA = psA_ctx.__enter__()
                with tc.tile_pool(name="psK0", bufs=1, space="PSUM") as psK:
                    pss = [
                        psK.tile([P, FT], f32, name=f"psk0_{eb}")
                        for eb in range(DB // 2)
                    ]
                    # Pass 1 (ebs 0-3) db-outer: first matmul needs only
                    # wk0+x0_0 (4 concurrent PSUM accumulation groups).
                    for db in range(DB):
                        for eb in range(DB // 2):
                            nc.tensor.matmul(
                                pss[eb],
                                lhsT=wk_slice(db, eb),
                                rhs=x0_ts[db],
                                start=(db == 0),
                                stop=(db == DB - 1),
                            )
                    for eb in range(DB // 2):
                        nc.scalar.activation(
                            out=kth_t[:, eb, 0:FT], in_=pss[eb],
                            func=AF.Identity, scale=SK,
                        )
                # Pass 2 (ebs 4-7) eb-outer on the psA rotation.
                for eb in range(DB // 2, DB):
                    ps = psA.tile([P, FT], f32, tag="ps")
                    for db in range(DB):
                        nc.tensor.matmul(
                            ps,
                            lhsT=wk_slice(db, eb),
                            rhs=x0_ts[db],
                            start=(db == 0),
                            stop=(db == DB - 1),
                        )
                    nc.scalar.activation(
                        out=kth_t[:, eb, 0:FT], in_=ps,
                        func=AF.Identity, scale=SK,
                    )

                # K projection n1 -> second K chunk gather
                for eb in range(DB):
                    ps = psA.tile([P, FT], f32, tag="ps")
                    for db in range(DB):
                        nc.tensor.matmul(
                            ps,
                            lhsT=wk_slice(db, eb),
                            rhs=x1_ts[db],
                            start=(db == 0),
                            stop=(db == DB - 1),
                        )
                    nc.scalar.activation(
                        out=kth_t[:, eb, FT:NQ], in_=ps,
                        func=AF.Identity, scale=SK,
                    )
                kth_dr = kth_d.rearrange("(db p) n -> p db n", p=P)
                nc.gpsimd.dma_start(out=kth_dr[:, :, :], in_=kth_t[:, :, :])
                nc.gpsimd.collective_compute(
                    "AllGather",
                    ALU.bypass,
                    replica_groups=PAIRS,
                    ins=[kth_d[:, :]],
                    outs=[ktg_d[:, :, :]],
                )

                # VW projection (own half) -> staging -> DRAM -> AllGather.
                # Last on the serial cc stream: with the fused output
                # matmul, VW is first consumed ~34us after st starts --
                # the K chunk gathers must land first (st(1) stalls 10.6us
                # if the 34us VW gather cuts ahead of K chunk B).
                for ks in range(KBH):
                    nt, kso = divmod(ks, FT // P)
                    for et in range(ET):
                        esl = slice(et * FT, (et + 1) * FT)
                        ps = psA.tile([P, FT], f32, tag="ps")
                        for db in range(DB):
                            nc.tensor.matmul(
                                ps,
                                lhsT=x_ts[nt][db][:, kso * P : (kso + 1) * P],
                                rhs=wvo_t[:, db, esl],
                                start=(db == 0),
                                stop=(db == DB - 1),
                            )
                        nc.any.tensor_copy(out=vh_t[:, ks, esl], in_=ps)
                # VW exchanged as two e-column chunk AllGathers: the fused
                # output matmul consumes e-blocks in order, so chunk A
                # (e 0:512) unblocks its first four chains ~17us earlier.
                vha_dr = vha_d.rearrange("(kb p) e -> p kb e", p=P)
                nc.gpsimd.dma_start(out=vha_dr[:, :, :], in_=vh_t[:, :, 0:FT])
                nc.gpsimd.collective_compute(
                    "AllGather",
                    ALU.bypass,
                    replica_groups=PAIRS,
                    ins=[vha_d[:, :]],
                    outs=[vga_d[:, :, :]],
                )
                vhb_dr = vhb_d.rearrange("(kb p) e -> p kb e", p=P)
                nc.gpsimd.dma_start(out=vhb_dr[:, :, :], in_=vh_t[:, :, FT:D])
                nc.gpsimd.collective_compute(
                    "AllGather",
                    ALU.bypass,
                    replica_groups=PAIRS,
                    ins=[vhb_d[:, :]],
                    outs=[vgb_d[:, :, :]],
                )
                # Gather loads stay on the sync queue, after the mask in
                # program order. (gpsimd was tried: collective_compute
                # blocks that queue until the serial cc stream accepts it,
                # so loads behind a trigger inherit the previous gather's
                # latency -- 13us st stall. The sync queue is kept free
                # during the first output phase by routing its stores to
                # the Activation queue instead.)
                for g in range(2):
                    kg_gr = ktg_d[g].rearrange("(db p) n -> p db n", p=P)
                    nc.sync.dma_start(
                        out=KT[:, :, g * NQ : (g + 1) * NQ],
                        in_=kg_gr[:, :, :],
                    )
                for et, vg in enumerate([vga_d, vgb_d]):
                    esl = slice(et * FT, (et + 1) * FT)
                    for g in range(2):
                        vg_gr = vg[g].rearrange("(kb p) e -> p kb e", p=P)
                        nc.sync.dma_start(
                            out=VW01[g][:, :, esl],
                            in_=vg_gr[:, :, :],
                        )

                # Q projection (overlaps the collectives)
                for nt in range(NTH):
                    nsl = slice(nt * FT, (nt + 1) * FT)
                    for eb in range(DB):
                        ps = psA.tile([P, FT], f32, tag="ps")
                        for db in range(DB):
                            nc.tensor.matmul(
                                ps,
                                lhsT=wq_t[:, db, eb * P : (eb + 1) * P],
                                rhs=x_ts[nt][db],
                                start=(db == 0),
                                stop=(db == DB - 1),
                            )
                        nc.scalar.activation(
                            out=QT[:, eb, nsl],
                            in_=ps,
                            func=AF.Identity,
                            bias=bq_t[:, eb : eb + 1],
                            scale=SQ,
                        )
                psA_ctx.__exit__(None, None, None)
                stage_ctx.__exit__(None, None, None)

            # ---- Phase B: scores+softmax, then fused output matmul ----
            with tc.tile_pool(name="persist2", bufs=1) as persist2:
                UT = persist2.tile([P, KB, NQ], bf16)
                s_sb = persist2.tile([1, NQ], f32)

                with (
                    tc.tile_pool(name="scr", bufs=4) as scr,
                    tc.tile_pool(name="psST", bufs=6, space="PSUM") as psST,
                    tc.tile_pool(name="psAV", bufs=2, space="PSUM") as psAV,
                ):
                    # Consume keys in K-gather arrival order:
                    # gather 0 -> kb {0-3, 8-11}, gather 1 -> kb {4-7, 12-15}
                    ST_KB_HALVES = [[0, 1, 2, 3, 8, 9, 10, 11],
                                    [4, 5, 6, 7, 12, 13, 14, 15]]

                    def st_half(half):
                        # Both q-tiles per kb so each DR stationary load
                        # (KT e-pair block) serves two moving matmuls.
                        for kb in ST_KB_HALVES[half]:
                            pss_qt = [
                                psST.tile([P, FT], f32, tag="st",
                                         name=f"ps_st{qt}_{kb}")
                                for qt in range(QT_TILES)
                            ]
                            for ep in range(DB // 2):
                                for qt in range(QT_TILES):
                                    qsl = slice(qt * FT, (qt + 1) * FT)
                                    nc.tensor.matmul(
                                        pss_qt[qt],
                                        lhsT=KT[:, 2 * ep : 2 * ep + 2,
                                                kb * P : (kb + 1) * P],
                                        rhs=QT[:, 2 * ep : 2 * ep + 2, qsl],
                                        start=(ep == 0),
                                        stop=(ep == DB // 2 - 1),
                                        perf_mode=DR,
                                    )
                            for qt in range(QT_TILES):
                                qsl = slice(qt * FT, (qt + 1) * FT)
                                sc = scr.tile([P, FT], f32, tag="sc",
                                              name=f"sc{qt}_{kb}")
                                nc.vector.tensor_tensor(
                                    sc, pss_qt[qt], maskfull[:, kb, qsl],
                                    op=ALU.add
                                )
                                nc.scalar.activation(
                                    out=UT[:, kb, qsl], in_=sc, func=AF.Exp,
                                    scale=EXP_SCALE,
                                )

                    def sums(qt):
                        # Row sums via ones-vector matmul over UT's key
                        # partitions. (A DVE adder-tree variant was tried:
                        # the scheduler hoists these matmuls ahead of the
                        # output chains and the PE then stalls 19.8us on
                        # the 18us serial tree. UT is ready by construction,
                        # so the plain version can never stall.)
                        qsl = slice(qt * FT, (qt + 1) * FT)
                        pss = psST.tile([1, FT], f32, tag="st",
                                        name=f"pss{qt}")
                        for kb in range(KB):
                            nc.tensor.matmul(
                                pss,
                                lhsT=ones_kb,
                                rhs=UT[:, kb, qsl],
                                start=(kb == 0),
                                stop=(kb == KB - 1),
                            )
                        nc.vector.tensor_copy(out=s_sb[:, qsl], in_=pss)

                    def avw(qt):
                        # Fused unnormalized output: outT = VW^T-blocks @ UT.
                        # Normalization happens on the host (srow output).
                        qsl = slice(qt * FT, (qt + 1) * FT)
                        for eb in range(DB):
                            ps = psAV.tile([P, FT], f32, tag="av",
                                           name=f"ps_av{qt}_{eb}")
                            for kb in range(KB):
                                nc.tensor.matmul(
                                    ps,
                                    lhsT=VW01[kb // KBH][:, kb % KBH,
                                                        eb * P : (eb + 1) * P],
                                    rhs=UT[:, kb, qsl],
                                    start=(kb == 0),
                                    stop=(kb == KB - 1),
                                )
                            ot = scr.tile([P, FT], f16, tag="ot")
                            nc.scalar.activation(
                                out=ot, in_=ps, func=AF.Identity, scale=1.0,
                            )
                            # qt0 stores all go to the Activation queue so
                            # the sync queue is empty when the gated VWb
                            # loads fire mid-avw(0); qt1 uses sync.
                            eng = nc.scalar if qt == 0 else nc.sync
                            qsl_o = slice(qt * FT, (qt + 1) * FT)
                            eng.dma_start(
                                out=outT[eb * P : (eb + 1) * P, qsl_o],
                                in_=ot,
                            )

                    # Order: the DVE tree fills during avw(0); the two sums
                    # passes slot between the output chains; srow's store
                    # issues before avw(1)'s drains hit the scalar queue.
                    st_half(0)
                    st_half(1)
                    avw(0)
                    avw(1)
                    sums(0)
                    sums(1)
                    nc.scalar.dma_start(out=srow, in_=s_sb)

    nc.compile()
    return nc


def _get_program():
    global _cached
    if _cached is None:
        _cached = _build_program()
    return _cached


def make_in_maps(x, mask, Wq, bq, Wk, bk, Wv, bv, Wo, bo):
    """Host-side preprocessing: per-core input dicts + bo_eff."""
    wq_h = Wq.astype(BF16H)
    wk_h = Wk.astype(BF16H)
    wvo_h = (Wv.astype(np.float32) @ Wo.astype(np.float32)).astype(BF16H)
    bq_s = (bq.astype(np.float32) * np.float32(SQ)).astype(np.float32)
    bo_eff = (
        bo.astype(np.float64) + bv.astype(np.float64) @ Wo.astype(np.float64)
    ).astype(np.float32)

    in_maps = []
    for c in range(8):
        b, h = divmod(c, 2)
        qs = slice(h * NQ, (h + 1) * NQ)
        xTh_c = np.ascontiguousarray(x[b, qs].T).astype(BF16H)  # [D, NQ]
        madd = np.where(
            mask[b, qs, :].T, np.float32(MASK_NEG), np.float32(0.0)
        ).astype(BF16H)  # [N, NQ], global key order
        in_maps.append(
            {
                "xTh": xTh_c,
                "maskadd": np.ascontiguousarray(madd),
                "wq": wq_h,
                "wk": wk_h,
                "wvo": wvo_h,
                "bq": bq_s,
            }
        )
    return in_maps, bo_eff


def assemble(results, bo_eff):
    out = np.empty((B, N, D), dtype=np.float32)
    for c in range(8):
        b, h = divmod(c, 2)
        s = results[c]["srow"].reshape(-1).astype(np.float32)  # [NQ]
        o = results[c]["outT"].T.astype(np.float32)            # [NQ, D]
        out[b, h * NQ : (h + 1) * NQ, :] = o / s[:, None] + bo_eff
    return out


def kernel(x, mask, Wq, bq, Wk, bk, Wv, bv, Wo, bo):
    from concourse.bass_utils import run_bass_kernel_spmd

    nc = _get_program()
    x, mask, Wq, bq, Wk, bk, Wv, bv, Wo, bo = (
        np.asarray(a) for a in (x, mask, Wq, bq, Wk, bk, Wv, bv, Wo, bo)
    )
    in_maps, bo_eff = make_in_maps(x, mask, Wq, bq, Wk, bk, Wv, bv, Wo, bo)
    res = run_bass_kernel_spmd(nc, in_maps, list(range(8)))
    return assemble(res.results, bo_eff)
